# revision 9
# baseline (speedup 1.0000x reference)
"""Trainium2 kernel for nn_LinearRowShared4Bit: out = x @ W.T + bias where W is
dequantized from 4-bit packed weights with per-(16-row-group, 32-col-block)
fp16 norms.

8-core tensor-parallel over out_features (1024 rows/core). Strategy: the host
dequantizes W exactly, scales by 8, and quantizes to TRN fp8 e3m4 (float8e3,
1-3-4, bias 3) — 4 mantissa bits give ~1.3e-2 rel l2 error, verified bit-exact
on HW (no subnormal flush). The device runs a pure streaming matmul:

  - Weights ship as e3m4 bytes in transposed layout [128 i-part, chunk*1024 o]
    (8.4 MB/core, plain contiguous DMA). Slice sizes ramp 1,1,2,4,8.. chunks so
    the first matmul starts early; all slices stay resident (no rotation).
  - lhsT = x.T/8 in fp16 [128, 16] per chunk (P=16 LDWEIGHTS); rhs = fp8
    columns. 2x column-tiling: o[0:512] runs on PE column group 0, o[512:1024]
    on group 1 (tile_position=(0,32)) — the two N=512 streams run concurrently,
    halving PE time to ~14us so the kernel is DMA-bound.
  - PE pre-warm: dummy matmuls on a memset tile during the DMA head keep HAM
    at K=8/8 before the real stream starts.
  - bias joins via a K=1 matmul that closes each accumulation group; drain is
    two parallel copies (DVE + ACT) then one output DMA.

Host gathers per-core [16, 1024] outputs -> [16, 8192].
"""

import numpy as np

O, I = 8192, 8192
GROUP, SHARE = 32, 16
NCORES = 8
OS = O // NCORES          # 1024 out rows per core
NCHUNK = I // 128         # 64 contraction chunks of 128
T_BATCH = 16
WSCALE = 8.0              # global power-of-2 scale into e3m4's normal range
SLICES = [1, 1, 2, 2, 4, 4, 6, 8, 8, 8, 8, 8, 2, 1, 1]  # chunks/slice, sum 64
NWARM = 13                # PE pre-warm matmuls (N=512)

_cache = {}


def _build_program():
    import concourse.mybir as mybir
    from concourse import bacc
    from concourse.tile import TileContext

    f16, f32, u8 = mybir.dt.float16, mybir.dt.float32, mybir.dt.uint8
    fp8 = mybir.dt.float8e3
    nc = bacc.Bacc("TRN2", target_bir_lowering=False, debug=False)

    wq_d = nc.dram_tensor("wq8", [128, NCHUNK * 1024], u8, kind="ExternalInput")
    xT_d = nc.dram_tensor("xT", [128, NCHUNK * T_BATCH], f16,
                          kind="ExternalInput")
    bias_d = nc.dram_tensor("biasf", [1, OS], f16, kind="ExternalInput")
    out_d = nc.dram_tensor("out", [T_BATCH, OS], f32, kind="ExternalOutput")

    with TileContext(nc) as tc:
        with (
            tc.tile_pool(name="const", bufs=1) as const,
            tc.tile_pool(name="ps", bufs=1, space="PSUM") as ps,
        ):
            # PE pre-warm: no-dependency matmuls on a memset tile
            wz = const.tile([128, 512], f16)
            nc.vector.memset(wz[:], 0.0)
            psW = ps.tile([T_BATCH, 512], f32)
            for _ in range(NWARM):
                nc.tensor.matmul(psW[:], wz[:, 0:T_BATCH], wz[:],
                                 start=True, stop=True)

            # x, bias on the ACT HWDGE ring (parallel to weight ring)
            xc = const.tile([128, NCHUNK * T_BATCH], f16)
            nc.scalar.dma_start(xc[:], xT_d[:])
            bf = const.tile([1, OS], f16)
            nc.scalar.dma_start(bf[:], bias_d[:])
            one1 = const.tile([1, T_BATCH], f16)
            nc.vector.memset(one1[:], 1.0)
            xv = xc[:].rearrange("p (c t) -> p c t", t=T_BATCH)

            # psum: group 0 -> partitions 0:16 (o 0:512), group 1 -> 32:48
            psU = ps.tile([48, 512], f32)
            psV = ps.tile([48, 512], f32)

            c0 = 0
            for s, ns in enumerate(SLICES):
                wt = const.tile([128, ns * 1024], u8, name=f"w{s}")
                eng = nc.sync if s % 2 == 0 else nc.scalar
                eng.dma_start(
                    wt[:], wq_d[:, c0 * 1024:(c0 + ns) * 1024])
                for j in range(ns):
                    c = c0 + j
                    lhs = xv[:, c, :]
                    rhs = wt[:, 1024 * j:1024 * (j + 1)].bitcast(fp8)
                    nc.tensor.matmul(psU[0:T_BATCH, :], lhs, rhs[:, 0:512],
                                     start=(c == 0), stop=False,
                                     tile_position=(0, 0))
                    nc.tensor.matmul(psV[32:32 + T_BATCH, :], lhs,
                                     rhs[:, 512:1024],
                                     start=(c == 0), stop=False,
                                     tile_position=(0, 32))
                c0 += ns
                # keep-warm only in the early region where DMA lags PE;
                # late dummies would delay the real matmul tail (measured)
                nwarm = {2: 6, 3: 10, 4: 10, 5: 4}.get(s, 0)
                for _ in range(nwarm):
                    nc.tensor.matmul(psW[:], wz[:, 0:T_BATCH], wz[:],
                                     start=True, stop=True)

            # bias closes both accumulation groups (K=1 matmul)
            nc.tensor.matmul(psU[0:T_BATCH, :], one1[:], bf[:, 0:512],
                             start=False, stop=True, tile_position=(0, 0))
            nc.tensor.matmul(psV[32:32 + T_BATCH, :], one1[:],
                             bf[:, 512:1024],
                             start=False, stop=True, tile_position=(0, 32))

            out_sb = const.tile([T_BATCH, OS], f32)
            nc.vector.tensor_copy(out_sb[:, 0:512], psU[0:T_BATCH, :])
            nc.scalar.copy(out_sb[:, 512:1024], psV[32:32 + T_BATCH, :])
            nc.sync.dma_start(out_d[:, 0:512], out_sb[:, 0:512])
            nc.scalar.dma_start(out_d[:, 512:1024], out_sb[:, 512:1024])

    nc.finalize()
    return nc


def _e3m4_grid():
    """Sorted finite positive e3m4 values with their byte encodings."""
    vals, bts = [], []
    for b in range(0x70):          # exp 0..6, positive
        e, m = (b >> 4) & 7, b & 15
        v = (m / 16.0) * 2.0 ** (-2) if e == 0 else (1 + m / 16.0) * 2.0 ** (e - 3)
        vals.append(v)
        bts.append(b)
    return np.array(vals), np.array(bts, np.uint8)


_GRID_V, _GRID_B = _e3m4_grid()


def _encode_e3m4(w):
    """w (any shape, |w| <= 15.5) -> nearest-value e3m4 bytes."""
    a = np.abs(w)
    idx = np.searchsorted(_GRID_V, a)
    idx = np.clip(idx, 1, len(_GRID_V) - 1)
    lo, hi = _GRID_V[idx - 1], _GRID_V[idx]
    pick = np.where(a - lo <= hi - a, idx - 1, idx)
    byte = _GRID_B[pick]
    byte = byte | np.where(np.signbit(w), np.uint8(0x80), np.uint8(0))
    return byte.astype(np.uint8)


def kernel(x, weight_q4, weight_norm, bias, _trace=False, _trace_kwargs=None):
    from concourse.bass_utils import run_bass_kernel_spmd

    if "nc" not in _cache:
        _cache["nc"] = _build_program()
    nc = _cache["nc"]

    # x.T/WSCALE in fp16, layout [partition p, chunk c, t] with i = 128c + p
    xs = (np.asarray(x, np.float64) / WSCALE).astype(np.float16)   # [16, I]
    xT = np.ascontiguousarray(
        xs.T.reshape(NCHUNK, 128, T_BATCH).transpose(1, 0, 2)
    ).reshape(128, NCHUNK * T_BATCH)

    in_maps = []
    for m in range(NCORES):
        wq = np.asarray(weight_q4[m * OS:(m + 1) * OS]).astype(np.uint8)
        low = wq & 15
        high = wq >> 4
        q8 = np.stack((low, high), axis=-1).reshape(OS, I // GROUP, GROUP)
        q8 = q8.astype(np.float32) / np.float32(15.0)
        norm = np.repeat(
            np.asarray(weight_norm[m * (OS // SHARE):(m + 1) * (OS // SHARE)],
                       np.float16).astype(np.float32), SHARE, axis=0)
        W = (q8 * np.float32(2.0) * norm - norm).reshape(OS, I)
        wb = _encode_e3m4(W.T * np.float32(WSCALE))      # [I, OS] bytes
        wb = np.ascontiguousarray(
            wb.reshape(NCHUNK, 128, OS).transpose(1, 0, 2)).reshape(128, -1)

        bb = np.asarray(bias[m * OS:(m + 1) * OS], np.float32).astype(
            np.float16).reshape(1, OS)
        in_maps.append(dict(wq8=wb, xT=xT, biasf=bb))

    res = run_bass_kernel_spmd(nc, in_maps, core_ids=list(range(NCORES)),
                               trace=_trace, **(_trace_kwargs or {}))
    outs = [r["out"] for r in res.results]
    full = np.concatenate(outs, axis=1).astype(np.float32)
    if _trace:
        return full, res
    return full


# revision 11
# speedup vs baseline: 1.0025x; 1.0025x over previous
"""Trainium2 kernel for nn_LinearRowShared4Bit: out = x @ W.T + bias where W is
dequantized from 4-bit packed weights with per-(16-row-group, 32-col-block)
fp16 norms.

8-core tensor-parallel over out_features (1024 rows/core). Strategy: the host
dequantizes W exactly, scales by 8, and quantizes to TRN fp8 e3m4 (float8e3,
1-3-4, bias 3) — 4 mantissa bits give ~1.3e-2 rel l2 error, verified bit-exact
on HW (no subnormal flush). The device runs a pure streaming matmul:

  - Weights ship as e3m4 bytes in transposed layout [128 i-part, chunk*1024 o]
    (8.4 MB/core, plain contiguous DMA). Slice sizes ramp 1,1,2,4,8.. chunks so
    the first matmul starts early; all slices stay resident (no rotation).
  - lhsT = x.T/8 in fp16 [128, 16] per chunk (P=16 LDWEIGHTS); rhs = fp8
    columns. 2x column-tiling: o[0:512] runs on PE column group 0, o[512:1024]
    on group 1 (tile_position=(0,32)) — the two N=512 streams run concurrently,
    halving PE time to ~14us so the kernel is DMA-bound.
  - PE pre-warm: dummy matmuls on a memset tile during the DMA head keep HAM
    at K=8/8 before the real stream starts.
  - bias joins via a K=1 matmul that closes each accumulation group; drain is
    two parallel copies (DVE + ACT) then one output DMA.

Host gathers per-core [16, 1024] outputs -> [16, 8192].
"""

import numpy as np

O, I = 8192, 8192
GROUP, SHARE = 32, 16
NCORES = 8
OS = O // NCORES          # 1024 out rows per core
NCHUNK = I // 128         # 64 contraction chunks of 128
T_BATCH = 16
WSCALE = 8.0              # global power-of-2 scale into e3m4's normal range
SLICES = [1, 1, 2, 2, 4, 4, 6] + [4] * 10 + [2, 1, 1]   # chunks/slice, sum 64
NWARM = 13                # PE pre-warm matmuls (N=512)

_cache = {}


def _build_program():
    import concourse.mybir as mybir
    from concourse import bacc
    from concourse.tile import TileContext

    f16, f32, u8 = mybir.dt.float16, mybir.dt.float32, mybir.dt.uint8
    fp8 = mybir.dt.float8e3
    nc = bacc.Bacc("TRN2", target_bir_lowering=False, debug=False)

    wq_d = nc.dram_tensor("wq8", [128, NCHUNK * 1024], u8, kind="ExternalInput")
    xT_d = nc.dram_tensor("xT", [128, NCHUNK * T_BATCH], f16,
                          kind="ExternalInput")
    bias_d = nc.dram_tensor("biasf", [1, OS], f16, kind="ExternalInput")
    out_d = nc.dram_tensor("out", [T_BATCH, OS], f32, kind="ExternalOutput")

    with TileContext(nc) as tc:
        with (
            tc.tile_pool(name="const", bufs=1) as const,
            tc.tile_pool(name="ps", bufs=1, space="PSUM") as ps,
        ):
            # PE pre-warm: no-dependency matmuls on a memset tile
            wz = const.tile([128, 512], f16)
            nc.vector.memset(wz[:], 0.0)
            psW = ps.tile([T_BATCH, 512], f32)
            for _ in range(NWARM):
                nc.tensor.matmul(psW[:], wz[:, 0:T_BATCH], wz[:],
                                 start=True, stop=True)

            # x, bias on the ACT HWDGE ring (parallel to weight ring)
            xc = const.tile([128, NCHUNK * T_BATCH], f16)
            nc.scalar.dma_start(xc[:], xT_d[:])
            bf = const.tile([1, OS], f16)
            nc.scalar.dma_start(bf[:], bias_d[:])
            one1 = const.tile([1, T_BATCH], f16)
            nc.vector.memset(one1[:], 1.0)
            xv = xc[:].rearrange("p (c t) -> p c t", t=T_BATCH)

            # psum: group 0 -> partitions 0:16 (o 0:512), group 1 -> 32:48
            psU = ps.tile([48, 512], f32)
            psV = ps.tile([48, 512], f32)

            c0 = 0
            for s, ns in enumerate(SLICES):
                wt = const.tile([128, ns * 1024], u8, name=f"w{s}")
                eng = nc.sync if s % 2 == 0 else nc.scalar
                eng.dma_start(
                    wt[:], wq_d[:, c0 * 1024:(c0 + ns) * 1024])
                for j in range(ns):
                    c = c0 + j
                    lhs = xv[:, c, :]
                    rhs = wt[:, 1024 * j:1024 * (j + 1)].bitcast(fp8)
                    nc.tensor.matmul(psU[0:T_BATCH, :], lhs, rhs[:, 0:512],
                                     start=(c == 0), stop=False,
                                     tile_position=(0, 0))
                    nc.tensor.matmul(psV[32:32 + T_BATCH, :], lhs,
                                     rhs[:, 512:1024],
                                     start=(c == 0), stop=False,
                                     tile_position=(0, 32))
                c0 += ns
                # keep-warm only in the early region where DMA lags PE;
                # late dummies would delay the real matmul tail (measured)
                if 2 <= s <= 4:
                    for _ in range(6):
                        nc.tensor.matmul(psW[:], wz[:, 0:T_BATCH], wz[:],
                                         start=True, stop=True)

            # bias closes both accumulation groups (K=1 matmul)
            nc.tensor.matmul(psU[0:T_BATCH, :], one1[:], bf[:, 0:512],
                             start=False, stop=True, tile_position=(0, 0))
            nc.tensor.matmul(psV[32:32 + T_BATCH, :], one1[:],
                             bf[:, 512:1024],
                             start=False, stop=True, tile_position=(0, 32))

            out_sb = const.tile([T_BATCH, OS], f32)
            nc.vector.tensor_copy(out_sb[:, 0:512], psU[0:T_BATCH, :])
            nc.scalar.copy(out_sb[:, 512:1024], psV[32:32 + T_BATCH, :])
            nc.sync.dma_start(out_d[:, 0:512], out_sb[:, 0:512])
            nc.scalar.dma_start(out_d[:, 512:1024], out_sb[:, 512:1024])

    nc.finalize()
    return nc


def _e3m4_grid():
    """Sorted finite positive e3m4 values with their byte encodings."""
    vals, bts = [], []
    for b in range(0x70):          # exp 0..6, positive
        e, m = (b >> 4) & 7, b & 15
        v = (m / 16.0) * 2.0 ** (-2) if e == 0 else (1 + m / 16.0) * 2.0 ** (e - 3)
        vals.append(v)
        bts.append(b)
    return np.array(vals), np.array(bts, np.uint8)


_GRID_V, _GRID_B = _e3m4_grid()


def _encode_e3m4(w):
    """w (any shape, |w| <= 15.5) -> nearest-value e3m4 bytes."""
    a = np.abs(w)
    idx = np.searchsorted(_GRID_V, a)
    idx = np.clip(idx, 1, len(_GRID_V) - 1)
    lo, hi = _GRID_V[idx - 1], _GRID_V[idx]
    pick = np.where(a - lo <= hi - a, idx - 1, idx)
    byte = _GRID_B[pick]
    byte = byte | np.where(np.signbit(w), np.uint8(0x80), np.uint8(0))
    return byte.astype(np.uint8)


def kernel(x, weight_q4, weight_norm, bias, _trace=False, _trace_kwargs=None):
    from concourse.bass_utils import run_bass_kernel_spmd

    if "nc" not in _cache:
        _cache["nc"] = _build_program()
    nc = _cache["nc"]

    # x.T/WSCALE in fp16, layout [partition p, chunk c, t] with i = 128c + p
    xs = (np.asarray(x, np.float64) / WSCALE).astype(np.float16)   # [16, I]
    xT = np.ascontiguousarray(
        xs.T.reshape(NCHUNK, 128, T_BATCH).transpose(1, 0, 2)
    ).reshape(128, NCHUNK * T_BATCH)

    in_maps = []
    for m in range(NCORES):
        wq = np.asarray(weight_q4[m * OS:(m + 1) * OS]).astype(np.uint8)
        low = wq & 15
        high = wq >> 4
        q8 = np.stack((low, high), axis=-1).reshape(OS, I // GROUP, GROUP)
        q8 = q8.astype(np.float32) / np.float32(15.0)
        norm = np.repeat(
            np.asarray(weight_norm[m * (OS // SHARE):(m + 1) * (OS // SHARE)],
                       np.float16).astype(np.float32), SHARE, axis=0)
        W = (q8 * np.float32(2.0) * norm - norm).reshape(OS, I)
        wb = _encode_e3m4(W.T * np.float32(WSCALE))      # [I, OS] bytes
        wb = np.ascontiguousarray(
            wb.reshape(NCHUNK, 128, OS).transpose(1, 0, 2)).reshape(128, -1)

        bb = np.asarray(bias[m * OS:(m + 1) * OS], np.float32).astype(
            np.float16).reshape(1, OS)
        in_maps.append(dict(wq8=wb, xT=xT, biasf=bb))

    res = run_bass_kernel_spmd(nc, in_maps, core_ids=list(range(NCORES)),
                               trace=_trace, **(_trace_kwargs or {}))
    outs = [r["out"] for r in res.results]
    full = np.concatenate(outs, axis=1).astype(np.float32)
    if _trace:
        return full, res
    return full
